# revision 25
# baseline (speedup 1.0000x reference)
"""Transformer block (LN->MHA->residual->LN->MLP->residual) on 8 trn2 cores.

Data-parallel over batch: each of the 8 NeuronCores processes one [1024, 768]
batch element with the full weight set.  No collectives.

v2 layout strategy per core:
  - token-major ([tokens, feat]) for LN stats, residuals, final output;
  - feature-major ([feat, tokens]) for matmul operands via PE transposes,
    batched per token tile into one PSUM bank and copied out in one ACT op;
  - attention runs ns(slab)-outer, head-pair inner, software-pipelined at
    matmul granularity: the two 64-dim head halves of each scores matmul are
    packed into PE row groups (0,0)/(64,0) so they run concurrently, land in
    the two halves of a [128,1024] two-bank PSUM tile, and are consumed by a
    single 1024-wide exp;
  - softmax denominator via a ones-slot in w_v (65th row of attn@v);
    normalization: fast approx reciprocal (DVE) + gpsimd partition broadcast
    + one DVE multiply;
  - ACT does only exp during attention (qkT copies on DVE), avoiding
    activation-table thrash; LN sqrts batched before any gelu;
  - proj for the first token half + all qkT q-tiles overlap attention;
    fc1/fc2 weight DMAs overlap proj/LN2/fc1 compute;
  - matmul operands bf16, accumulation + residual path fp32.
"""

import numpy as np

P = 128
N = 1024          # tokens per core
C = 768           # embed
H = 12            # heads
D = 64            # head dim
HID = 3072
NT = N // P       # 8 token tiles
CK = C // P       # 6 feature k-tiles
HK = HID // P     # 24
VW = H * (D + 1)  # 780 = v-aug width
EPS = 1e-5
NSLABS = 2
NSL = N // NSLABS  # 512
HP = H // 2        # 6 head pairs


def _emit(tc, io, gelu_mode="hw", mm_dt="bf16"):
    """Emit the whole block into TileContext tc. io: dict name->AP."""
    from contextlib import ExitStack

    import concourse.bass as bass
    from concourse import mybir
    from concourse.masks import make_identity

    nc = tc.nc
    fp32 = mybir.dt.float32
    bf16 = mybir.dt.bfloat16 if mm_dt == "bf16" else mybir.dt.float32
    AF = mybir.ActivationFunctionType
    ALU = mybir.AluOpType

    with ExitStack() as ctx:
        const = ctx.enter_context(tc.tile_pool(name="const", bufs=1))
        work = ctx.enter_context(tc.tile_pool(name="work", bufs=2))
        # 3 x [128,1024] fp32 (2 banks each) for matmul outputs
        psPair = ctx.enter_context(tc.tile_pool(name="psPair", bufs=3,
                                                space="PSUM"))
        # 2 x [128,512] fp32 banks: attnv accumulators + LN transpose staging
        psO = ctx.enter_context(tc.tile_pool(name="psO", bufs=2, space="PSUM"))

        # ---------- constants ----------
        ident = const.tile([P, P], bf16, tag="ident", name="ident")
        make_identity(nc, ident)

        eps_t = const.tile([P, 1], fp32, tag="eps", name="eps")
        nc.vector.memset(eps_t, EPS)

        def bcast_row(nm, width):
            """DMA a [width] dram row broadcast across 128 partitions."""
            t = const.tile([P, width], fp32, tag=f"bc_{nm}", name=f"bc_{nm}")
            src = io[nm]
            bc = bass.AP(tensor=src.tensor, offset=src.offset,
                         ap=[[0, P]] + list(src.ap))
            nc.gpsimd.dma_start(out=t, in_=bc)
            return t


        def col_bias(nm, width, q=None):
            """[width] dram -> [128, width//P] per-partition columns."""
            t = const.tile([P, width // P], fp32, tag=f"cb_{nm}", name=f"cb_{nm}")
            (q or nc.gpsimd).dma_start(
                out=t, in_=io[nm].rearrange("(o p) -> p o", p=P))
            return t


        mvpool = ctx.enter_context(tc.tile_pool(name="mvpool", bufs=1))

        def ln_stats(xt, t):
            """bn stats for tile xt -> persistent mv tile (mean, var)."""
            sub = 256
            nsub = C // sub
            stats = work.tile([P, nsub, 6], fp32, tag="ln_stats",
                              name="ln_stats")
            xg = xt.rearrange("p (s f) -> p s f", s=nsub)
            for s in range(nsub):
                nc.vector.bn_stats(out=stats[:, s, :], in_=xg[:, s, :])
            mv = mvpool.tile([P, 2], fp32, tag=f"mv_{t}", name=f"mv_{t}")
            nc.vector.bn_aggr(out=mv, in_=stats)
            return mv

        def ln_finish(xt, mv, dstT_col):
            """normalize + transpose into feature-major big-T columns."""
            std = work.tile([P, 1], fp32, tag="ln_std", name="ln_std")
            nc.scalar.activation(out=std, in_=mv[:, 1:2], func=AF.Sqrt,
                                 bias=eps_t, scale=1.0)
            istd = work.tile([P, 1], fp32, tag="ln_istd", name="ln_istd")
            nc.vector.reciprocal(out=istd, in_=std)
            hn = work.tile([P, C], bf16, tag="ln_hn", name="ln_hn")
            nc.vector.tensor_scalar(out=hn, in0=xt,
                                    scalar1=mv[:, 0:1], scalar2=istd,
                                    op0=ALU.subtract, op1=ALU.mult)
            pt = psO.tile([P, C], bf16, tag="o", name="pt")
            for c in range(CK):
                nc.tensor.transpose(pt[:, c * P:(c + 1) * P],
                                    hn[:, c * P:(c + 1) * P], ident)
            # one strided copy: [128, 768] psum -> CK column blocks of big T
            nc.scalar.copy(out=dstT_col, in_=pt)

        def ln_tile(xt, dstT_col, t):
            ln_finish(xt, ln_stats(xt, t), dstT_col)

        # ============ pool lifetimes (manually managed, non-LIFO) ============
        x2pool = ctx.enter_context(tc.tile_pool(name="x2pool", bufs=1))
        xpool_cm = tc.tile_pool(name="xpool", bufs=1)
        wproj_cm = tc.tile_pool(name="wproj", bufs=1)
        aopool_cm = tc.tile_pool(name="aopool", bufs=1)
        xpool = xpool_cm.__enter__()
        wproj_pool = wproj_cm.__enter__()
        aopool = aopool_cm.__enter__()
        if True:
            x_sb = [xpool.tile([P, C], fp32, tag=f"x_{t}", name=f"x_{t}")
                    for t in range(NT)]
            x2_sb = [x2pool.tile([P, C], fp32, tag=f"x2_{t}", name=f"x2_{t}")
                     for t in range(NT)]
            attn_oT = [aopool.tile([P, N], bf16, tag=f"aoT_{c}",
                                   name=f"aoT_{c}") for c in range(CK)]
            w_proj_sb = []

            def emit_proj(t):
                """proj + residual for token tile t (x_sb already has b_proj
                folded in)."""
                ps = psPair.tile([P, 2 * NSL], fp32, tag="pair",
                                 name="pair")[:, :C]
                for off, w in ((0, 512), (512, 256)):
                    nc.tensor.matmul(
                        ps[:, off:off + w], ones_col,
                        b_proj_bf[:, off:off + w],
                        start=True, stop=False)
                    for k in range(CK):
                        nc.tensor.matmul(
                            ps[:, off:off + w],
                            attn_oT[k][:, t * P:(t + 1) * P],
                            w_proj_sb[k][:, off:off + w],
                            start=False, stop=(k == CK - 1))
                nc.vector.tensor_add(out=x2_sb[t], in0=ps, in1=x_sb[t])

            with tc.tile_pool(name="h1pool", bufs=1) as h1pool, \
                 tc.tile_pool(name="qkpool", bufs=1) as qkpool, \
                 tc.tile_pool(name="vpool", bufs=1) as vpool, \
                 tc.tile_pool(name="wqk", bufs=1) as wqk_pool, \
                 tc.tile_pool(name="wv", bufs=1) as wv_pool, \
                 tc.tile_pool(name="epool", bufs=2) as epool, \
                 tc.tile_pool(name="rpool", bufs=2) as rpool:

                h1T = h1pool.tile([P, CK, N], bf16, tag="h1T", name="h1T")
                qkT = qkpool.tile([P, 2 * CK, N], bf16, tag="qkT", name="qkT")
                v_sb = [vpool.tile([P, VW], bf16, tag=f"v_{t}",
                                   name=f"v_{t}") for t in range(NT)]

                # -------- DMAs: x on sync queue; gpsimd queue in priority
                # order w_v -> w_qk -> biases -> w_proj --------
                for t in range(4):
                    nc.sync.dma_start(out=x_sb[t],
                                      in_=io["x"][t * P:(t + 1) * P, :])
                for t in range(4, NT):
                    nc.scalar.dma_start(out=x_sb[t],
                                        in_=io["x"][t * P:(t + 1) * P, :])
                ones_col = const.tile([1, P], bf16, tag="ones_col",
                                      name="ones_col")
                nc.vector.memset(ones_col, 1.0)
                b_qk_col = col_bias("b_qk", 2 * C)      # [128, 12]
                b_v_bf = const.tile([1, VW], bf16, tag="b_v_bf",
                                    name="b_v_bf")
                nc.gpsimd.dma_start(out=b_v_bf, in_=io["b_v_bf"])
                b_proj_bf = const.tile([1, C], bf16, tag="b_proj_bf",
                                       name="b_proj_bf")
                nc.gpsimd.dma_start(out=b_proj_bf, in_=io["b_proj_bf"])
                w_v_sb = []
                for k in range(CK):
                    wt = wv_pool.tile([P, VW], bf16, tag=f"wv_{k}",
                                      name=f"wv_{k}")
                    nc.gpsimd.dma_start(
                        out=wt, in_=io["w_v_aug"][k * P:(k + 1) * P, :])
                    w_v_sb.append(wt)
                w_qk_sb = []
                for k in range(CK):
                    wt = wqk_pool.tile([P, 2 * C], bf16,
                                       tag=f"wqk_{k}", name=f"wqk_{k}")
                    nc.gpsimd.dma_start(
                        out=wt, in_=io["w_qk"][k * P:(k + 1) * P, :])
                    w_qk_sb.append(wt)
                b_fc2_bc = bcast_row("b_fc2", C)        # [128, 768]
                for k in range(CK):
                    wt = wproj_pool.tile([P, C], bf16, tag=f"wp_{k}",
                                         name=f"wp_{k}")
                    nc.gpsimd.dma_start(
                        out=wt, in_=io["w_proj"][k * P:(k + 1) * P, :])
                    w_proj_sb.append(wt)
                b_fc1_col = col_bias("b_fc1", HID)      # [128, 24]

                def emit_v(t, on_act=False):
                    ps = psPair.tile([P, 2 * NSL], fp32, tag="pair",
                                     name="pair")[:, :VW]
                    for off, w in ((0, 512), (512, VW - 512)):
                        # K=1 preload seeds the bias row (incl. the
                        # denominator ones-slots), then accumulate
                        nc.tensor.matmul(
                            ps[:, off:off + w], ones_col,
                            b_v_bf[:, off:off + w],
                            start=True, stop=False)
                        for k in range(CK):
                            nc.tensor.matmul(
                                ps[:, off:off + w],
                                h1T[:, k, t * P:(t + 1) * P],
                                w_v_sb[k][:, off:off + w],
                                start=False, stop=(k == CK - 1))
                    if on_act:
                        nc.scalar.copy(out=v_sb[t], in_=ps)
                    else:
                        nc.vector.tensor_copy(out=v_sb[t], in_=ps)

                def emit_qk_block(mpair, ns, on_act=False):
                    """qkT m-tiles (mpair, mpair+1) for slab ns: 12 MMs into a
                    [128,1024] pair + two biased copies."""
                    ps = psPair.tile([P, 2 * NSL], fp32, tag="pair",
                                     name="pair")
                    nsl = slice(ns * NSL, (ns + 1) * NSL)
                    for mi in range(2):
                        m = mpair + mi
                        for k in range(CK):
                            nc.tensor.matmul(
                                ps[:, mi * NSL:(mi + 1) * NSL],
                                w_qk_sb[k][:, m * P:(m + 1) * P],
                                h1T[:, k, nsl],
                                start=(k == 0), stop=(k == CK - 1))
                    for mi in range(2):
                        m = mpair + mi
                        if on_act:
                            nc.scalar.activation(
                                out=qkT[:, m, nsl],
                                in_=ps[:, mi * NSL:(mi + 1) * NSL],
                                func=AF.Identity,
                                bias=b_qk_col[:, m:m + 1], scale=1.0)
                        else:
                            nc.vector.tensor_scalar_add(
                                out=qkT[:, m, nsl],
                                in0=ps[:, mi * NSL:(mi + 1) * NSL],
                                scalar1=b_qk_col[:, m:m + 1])

                # ---------- P1: LN1; v/qk fills the w_qk DMA window ----
                for t in range(4):
                    ln_tile(x_sb[t], h1T[:, :, t * P:(t + 1) * P], t)
                for t in range(4):
                    emit_v(t, on_act=True)
                ln_tile(x_sb[4], h1T[:, :, 4 * P:5 * P], 4)
                emit_v(4, on_act=True)
                for t in range(5, NT):
                    ln_tile(x_sb[t], h1T[:, :, t * P:(t + 1) * P], t)
                emit_qk_block(CK, 0, on_act=True)   # k m=6,7 keys 0:512
                emit_qk_block(0, 0, on_act=True)    # q m=0,1 queries 0:512

                # ---------- attention: ns-outer, software-pipelined ----------
                slabs = [(hp, ns) for ns in range(NSLABS) for hp in range(HP)]
                E_tiles = {}   # live E tiles per slab index
                ps_o = {}      # psum accumulators per slab index

                def emit_scores_mt(i, mt):
                    hp, ns = slabs[i]
                    nsl = slice(ns * NSL, (ns + 1) * NSL)
                    ps = psPair.tile([P, 2 * NSL], fp32, tag="pair",
                                     name="pair")
                    for half, pr in ((0, slice(0, D)), (1, slice(D, P))):
                        nc.tensor.matmul(
                            ps[:, half * NSL:(half + 1) * NSL],
                            qkT[pr, CK + hp, mt * P:(mt + 1) * P],
                            qkT[pr, hp, nsl],
                            start=True, stop=True,
                            tile_position=(half * D, 0))
                    e = epool.tile([P, 2 * NSL], bf16, tag=f"E_{mt}",
                                   name=f"E_{mt}")
                    nc.scalar.activation(out=e, in_=ps, func=AF.Exp,
                                         scale=0.125)
                    E_tiles[i] = E_tiles.get(i, {})
                    E_tiles[i][mt] = e

                def emit_attnv_mt(j, mt):
                    hp, ns = slabs[j]
                    if mt == 0:
                        ps_o[j] = [psO.tile([P, NSL], fp32, tag="o",
                                            name="o")[:D + 1, :]
                                   for _ in range(2)]
                    e = E_tiles[j][mt]
                    for half in range(2):
                        h = 2 * hp + half
                        nc.tensor.matmul(
                            ps_o[j][half],
                            v_sb[mt][:, h * (D + 1):(h + 1) * (D + 1)],
                            e[:, half * NSL:(half + 1) * NSL],
                            start=(mt == 0), stop=(mt == NT - 1))

                def emit_norm(j):
                    hp, ns = slabs[j]
                    nsl = slice(ns * NSL, (ns + 1) * NSL)
                    for half in range(2):
                        dsb = rpool.tile([1, NSL], fp32, tag="dsb",
                                         name="dsb")
                        nc.vector.tensor_copy(out=dsb,
                                              in_=ps_o[j][half][D:D + 1, :])
                        r = rpool.tile([1, NSL], fp32, tag="r", name="r")
                        nc.vector.reciprocal_approx_fast(out=r, in_=dsb)
                        rb = rpool.tile([D, NSL], fp32, tag="rb", name="rb")
                        nc.gpsimd.partition_broadcast(rb, r)
                        if "dbg_r" in io and j == len(slabs) - 1 and half == 0:
                            num2 = rpool.tile([2, NSL], fp32, tag="r",
                                              name="num2")
                            nc.vector.tensor_copy(out=num2,
                                                  in_=ps_o[j][half][:2, :])
                            den = rpool.tile([1, NSL], fp32, tag="r",
                                             name="den")
                            nc.vector.tensor_copy(out=den,
                                                  in_=ps_o[j][half][D:D + 1, :])
                            nc.sync.dma_start(out=io["dbg_r"], in_=r)
                            nc.sync.dma_start(out=io["dbg_den"], in_=den)
                            nc.sync.dma_start(out=io["dbg_num2"], in_=num2)
                            nc.sync.dma_start(out=io["dbg_rb"], in_=rb[0:1, :])
                            nc.sync.dma_start(out=io["dbg_e"],
                                              in_=E_tiles[j][0])
                        nc.vector.tensor_mul(
                            out=attn_oT[hp][half * D:(half + 1) * D, nsl],
                            in0=ps_o[j][half][:D, :], in1=rb)
                    del E_tiles[j]
                    del ps_o[j]

                mv_saved = {}

                def emit_proj_split(t):
                    state = {}

                    def chunkA():
                        ps = psPair.tile([P, 2 * NSL], fp32, tag="pair",
                                         name="pair")[:, :C]
                        state["ps"] = ps
                        nc.tensor.matmul(ps[:, :512], ones_col,
                                         b_proj_bf[:, :512],
                                         start=True, stop=False)
                        for k in range(CK):
                            nc.tensor.matmul(
                                ps[:, :512],
                                attn_oT[k][:, t * P:(t + 1) * P],
                                w_proj_sb[k][:, :512],
                                start=False, stop=(k == CK - 1))

                    def chunkB():
                        ps = state["ps"]
                        nc.tensor.matmul(ps[:, 512:], ones_col,
                                         b_proj_bf[:, 512:],
                                         start=True, stop=False)
                        for k in range(CK):
                            nc.tensor.matmul(
                                ps[:, 512:],
                                attn_oT[k][:, t * P:(t + 1) * P],
                                w_proj_sb[k][:, 512:],
                                start=False, stop=(k == CK - 1))
                        nc.vector.tensor_add(out=x2_sb[t], in0=ps,
                                             in1=x_sb[t])
                        mv_saved[t] = ln_stats(x2_sb[t], t)
                    return chunkA, chunkB

                proj_chunks = {t: emit_proj_split(t) for t in range(4)}

                # filler work blocks placed mid-slab (after the given mt) so
                # the exp stream stays fed; deps documented per entry
                fillers = {
                    0: {1: [lambda: emit_qk_block(CK, 1)],
                        3: [lambda: emit_v(5)],
                        5: [lambda: emit_qk_block(CK + 2, 0)],
                        6: [lambda: emit_qk_block(2, 0)]},
                    1: {0: [lambda: emit_v(6)],
                        1: [lambda: emit_v(7)],
                        5: [lambda: emit_qk_block(CK + 2, 1)]},
                    2: {4: [lambda: emit_qk_block(CK + 4, 0)],
                        5: [lambda: emit_qk_block(CK + 4, 1)]},
                    3: {5: [lambda: emit_qk_block(4, 0)]},
                    5: {5: [lambda: emit_qk_block(0, 1)]},
                    7: {3: [lambda: emit_qk_block(2, 1)],
                        4: [proj_chunks[0][0]],
                        6: [proj_chunks[0][1]]},
                    8: {4: [proj_chunks[1][0]],
                        6: [proj_chunks[1][1]]},
                    9: {2: [lambda: emit_qk_block(4, 1)],
                        4: [proj_chunks[2][0]],
                        6: [proj_chunks[2][1]]},
                    10: {4: [proj_chunks[3][0]],
                        6: [proj_chunks[3][1]]},
                }
                # attnv for the previous slab is compressed into positions
                # 0-3 (two mt per position) so its normalization chain can
                # run on DVE/gpsimd during positions 4-7 instead of
                # colliding with the next slab's boundary
                last = len(slabs) - 1
                for i in range(len(slabs)):
                    fill = fillers.get(i, {})
                    for mt in range(NT):
                        emit_scores_mt(i, mt)
                        if i > 0 and mt < 4:
                            emit_attnv_mt(i - 1, 2 * mt)
                            emit_attnv_mt(i - 1, 2 * mt + 1)
                        if i > 0 and mt == 3:
                            emit_norm(i - 1)
                        if i == last and mt >= 4:
                            # drain the final slab's attnv in-slab (each
                            # E tile is written earlier in this position)
                            emit_attnv_mt(last, 2 * (mt - 4))
                            emit_attnv_mt(last, 2 * (mt - 4) + 1)
                        for fn in fill.get(mt, []):
                            fn()
                emit_norm(last)

                if "dbg_h1T" in io:
                    nc.sync.dma_start(out=io["dbg_h1T"],
                                      in_=h1T.rearrange("p c n -> p (c n)"))
                    nc.sync.dma_start(out=io["dbg_qkT"],
                                      in_=qkT.rearrange("p c n -> p (c n)"))
                    nc.sync.dma_start(out=io["dbg_v0"], in_=v_sb[0])
                    nc.sync.dma_start(out=io["dbg_ao"], in_=attn_oT[0])

            # ---------- attention pools closed; proj t4-7, LN2, MLP ----------
            w1pool = ctx.enter_context(tc.tile_pool(name="w1pool", bufs=1, side="right"))
            h2pool = ctx.enter_context(tc.tile_pool(name="h2pool", bufs=1, side="right"))
            if True:
                w1 = []
                for k in range(CK):
                    wt = w1pool.tile([P, HID], bf16, tag=f"wfc1_{k}",
                                     name=f"wfc1_{k}")
                    nc.sync.dma_start(
                        out=wt, in_=io["w_fc1"][k * P:(k + 1) * P, :])
                    w1.append(wt)
                h2T = h2pool.tile([P, CK, N], bf16, tag="h2T", name="h2T")

                # proj t4-7 interleaved with LN2-finish of t0-3 (stats for
                # t0-3 were computed during attention)
                for t in range(4, NT):
                    emit_proj(t)
                    t2 = t - 4
                    ln_finish(x2_sb[t2], mv_saved[t2],
                              h2T[:, :, t2 * P:(t2 + 1) * P])

                # x / w_proj / attn_oT are dead now; free before MLP pools
                aopool_cm.__exit__(None, None, None)
                wproj_cm.__exit__(None, None, None)
                xpool_cm.__exit__(None, None, None)

                if "dbg_x2" in io:
                    nc.sync.dma_start(out=io["dbg_x2"], in_=x2_sb[0])
                    nc.sync.dma_start(out=io["dbg_bpf"], in_=b_proj_bf)
                    nc.sync.dma_start(out=io["dbg_bfc2"],
                                      in_=b_fc2_bc[0:1, :])
                    nc.sync.dma_start(
                        out=io["dbg_h2T"],
                        in_=h2T.rearrange("p c n -> p (c n)"))
                gelu_f = AF.Gelu if gelu_mode == "hw" else AF.Identity
                with tc.tile_pool(name="gpool", bufs=1, side="right") as gpool, \
                     tc.tile_pool(name="w2pool", bufs=1, side="right") as w2pool, \
                     tc.tile_pool(name="opool", bufs=3, side="right") as opool:
                    gT = gpool.tile([P, HK, N], bf16, tag="gT", name="gT")
                    w2 = []
                    for k in range(HK):
                        wt = w2pool.tile([P, C], bf16, tag=f"wfc2_{k}",
                                         name=f"wfc2_{k}")
                        nc.sync.dma_start(
                            out=wt, in_=io["w_fc2"][k * P:(k + 1) * P, :])
                        w2.append(wt)

                    def emit_fc1_block(mpair, ns, defer=False):
                        ps = psPair.tile([P, 2 * NSL], fp32, tag="pair",
                                         name="pair")
                        nsl = slice(ns * NSL, (ns + 1) * NSL)
                        for mi in range(2):
                            m = mpair + mi
                            for k in range(CK):
                                nc.tensor.matmul(
                                    ps[:, mi * NSL:(mi + 1) * NSL],
                                    w1[k][:, m * P:(m + 1) * P],
                                    h2T[:, k, nsl],
                                    start=(k == 0), stop=(k == CK - 1))

                        def flush():
                            for mi in range(2):
                                m = mpair + mi
                                nc.scalar.activation(
                                    out=gT[:, m, nsl],
                                    in_=ps[:, mi * NSL:(mi + 1) * NSL],
                                    func=gelu_f,
                                    bias=b_fc1_col[:, m:m + 1], scale=1.0)
                        if defer:
                            return flush
                        flush()
                        return None

                    # fc1 ns0 mpair0 runs with gelu deferred so the LN2 t4-7
                    # sqrts (different ACT table set) all precede the first
                    # gelu; its matmuls cover the LN2 tail on the PE
                    fls = [emit_fc1_block(mp, 0, defer=True)
                           for mp in (0, 2, 4)]
                    for t in range(4, NT):
                        ln_tile(x2_sb[t], h2T[:, :, t * P:(t + 1) * P], t)
                    for fl in fls:
                        fl()
                    for mp in range(6, HK, 2):
                        emit_fc1_block(mp, 0)
                    # x2 += b_fc2 for the fc2 residual; DVE is idle here
                    for t in range(NT):
                        nc.vector.tensor_add(out=x2_sb[t], in0=x2_sb[t],
                                             in1=b_fc2_bc)
                    for mp in range(0, HK, 2):
                        emit_fc1_block(mp, 1)

                    for t in range(NT):
                        ps = psPair.tile([P, 2 * NSL], fp32, tag="pair",
                                         name="pair")[:, :C]
                        for off, w in ((0, 512), (512, 256)):
                            for k in range(HK):
                                nc.tensor.matmul(
                                    ps[:, off:off + w],
                                    gT[:, k, t * P:(t + 1) * P],
                                    w2[k][:, off:off + w],
                                    start=(k == 0), stop=(k == HK - 1))
                        ot = opool.tile([P, C], fp32, tag="out_t",
                                        name="out_t")
                        nc.vector.tensor_add(out=ot, in0=ps, in1=x2_sb[t])
                        nc.sync.dma_start(
                            out=io["out"][t * P:(t + 1) * P, :], in_=ot)


def build_program(gelu_mode="hw", mm_dt="bf16", debug_io=False):
    import concourse.tile as tile
    from concourse import bacc, mybir

    fp32 = mybir.dt.float32
    bf16 = mybir.dt.bfloat16 if mm_dt == "bf16" else mybir.dt.float32
    nc = bacc.Bacc("TRN2", target_bir_lowering=False, debug=False,
                   num_devices=8)

    shapes = {
        "x": ([N, C], fp32),
        "w_qk": ([C, 2 * C], bf16), "b_qk": ([2 * C], fp32),
        "b_v_bf": ([VW], bf16), "b_proj_bf": ([C], bf16),
        "w_v_aug": ([C, VW], bf16), "b_v_aug": ([VW], fp32),
        "w_proj": ([C, C], bf16), "b_proj": ([C], fp32),
        "w_fc1": ([C, HID], bf16), "b_fc1": ([HID], fp32),
        "w_fc2": ([HID, C], bf16), "b_fc2": ([C], fp32),
    }
    io = {}
    for name, (shp, dt) in shapes.items():
        io[name] = nc.dram_tensor(name, shp, dt, kind="ExternalInput").ap()
    io["out"] = nc.dram_tensor("out", [N, C], fp32, kind="ExternalOutput").ap()
    if debug_io:
        for nm, shp in (("dbg_h1T", [P, CK * N]), ("dbg_qkT", [P, 2 * CK * N]),
                        ("dbg_v0", [P, VW]), ("dbg_ao", [P, N])):
            io[nm] = nc.dram_tensor(nm, shp, bf16,
                                    kind="ExternalOutput").ap()
        for nm, shp in (("dbg_r", [1, NSL]), ("dbg_den", [1, NSL]),
                        ("dbg_num2", [2, NSL]), ("dbg_rb", [1, NSL])):
            io[nm] = nc.dram_tensor(nm, shp, fp32,
                                    kind="ExternalOutput").ap()
        io["dbg_e"] = nc.dram_tensor("dbg_e", [P, N], bf16,
                                     kind="ExternalOutput").ap()
        io["dbg_x2"] = nc.dram_tensor("dbg_x2", [P, C], fp32,
                                      kind="ExternalOutput").ap()
        io["dbg_bpf"] = nc.dram_tensor("dbg_bpf", [1, C], bf16,
                                       kind="ExternalOutput").ap()
        io["dbg_bfc2"] = nc.dram_tensor("dbg_bfc2", [1, C], fp32,
                                        kind="ExternalOutput").ap()
        io["dbg_h2T"] = nc.dram_tensor("dbg_h2T", [P, CK * N], bf16,
                                       kind="ExternalOutput").ap()

    with tile.TileContext(nc) as tc:
        _emit(tc, io, gelu_mode=gelu_mode, mm_dt=mm_dt)
    nc.compile()
    return nc


def host_prep(inputs, mm_dt="bf16"):
    """Fold LN1/LN2 scale+shift into w_qk/w_v/w_fc1; build v-aug layout;
    cast weights to bf16."""
    import ml_dtypes
    f32 = np.float32
    bf = ml_dtypes.bfloat16 if mm_dt == "bf16" else np.float32

    x = np.asarray(inputs["x"], f32)
    w_qkv = np.asarray(inputs["w_qkv"], f32)
    b_qkv = np.asarray(inputs["b_qkv"], f32)
    ln1_w = np.asarray(inputs["ln1_w"], f32)
    ln1_b = np.asarray(inputs["ln1_b"], f32)
    ln2_w = np.asarray(inputs["ln2_w"], f32)
    ln2_b = np.asarray(inputs["ln2_b"], f32)

    w_q = w_qkv[:, 0:C]
    w_k = w_qkv[:, C:2 * C]
    w_v = w_qkv[:, 2 * C:3 * C]
    b_q = b_qkv[0:C]
    b_k = b_qkv[C:2 * C]
    b_v = b_qkv[2 * C:3 * C]

    # fold LN1: h = hn*ln1_w + ln1_b  =>  W' = ln1_w[:,None]*W, b' = b + W.T@ln1_b
    w_qk = np.concatenate([w_q, w_k], axis=1)          # [C, 2C]
    w_qk_f = ln1_w[:, None] * w_qk
    b_qk_f = np.concatenate([b_q, b_k]) + w_qk.T @ ln1_b

    w_v_f = ln1_w[:, None] * w_v
    b_v_f = b_v + w_v.T @ ln1_b
    w_v_aug = np.zeros((C, VW), f32)
    b_v_aug = np.zeros((VW,), f32)
    for h in range(H):
        w_v_aug[:, h * (D + 1):h * (D + 1) + D] = w_v_f[:, h * D:(h + 1) * D]
        b_v_aug[h * (D + 1) + D] = 1.0

    w_fc1 = np.asarray(inputs["w_fc1"], f32)
    b_fc1 = np.asarray(inputs["b_fc1"], f32)
    w_fc1_f = ln2_w[:, None] * w_fc1
    b_fc1_f = b_fc1 + w_fc1.T @ ln2_b

    common = {
        "w_qk": np.ascontiguousarray(w_qk_f).astype(bf),
        "b_v_bf": b_v_aug.astype(bf),
        "b_qk": np.ascontiguousarray(b_qk_f, f32),
        "w_v_aug": w_v_aug.astype(bf),
        "b_v_aug": b_v_aug,
        "w_proj": np.ascontiguousarray(np.asarray(inputs["w_proj"], f32)).astype(bf),
        # softmax rows sum to 1, so the v bias rides through attention
        # unchanged and folds into the proj bias
        "b_proj": np.ascontiguousarray(
            np.asarray(inputs["b_proj"], f32)
            + b_v_f @ np.asarray(inputs["w_proj"], f32)),
        "b_proj_bf": (np.asarray(inputs["b_proj"], f32)
                      + b_v_f @ np.asarray(inputs["w_proj"], f32)).astype(bf),
        "w_fc1": np.ascontiguousarray(w_fc1_f).astype(bf),
        "b_fc1": np.ascontiguousarray(b_fc1_f, f32),
        "w_fc2": np.ascontiguousarray(np.asarray(inputs["w_fc2"], f32)).astype(bf),
        "b_fc2": np.ascontiguousarray(np.asarray(inputs["b_fc2"], f32)),
    }
    in_maps = []
    for i in range(x.shape[0]):
        m = dict(common)
        m["x"] = np.ascontiguousarray(x[i])
        in_maps.append(m)
    return in_maps


_CACHE = {}


def kernel(**inputs):
    from concourse.bass_utils import run_bass_kernel_spmd

    if "nc" not in _CACHE:
        _CACHE["nc"] = build_program(gelu_mode="hw")
    nc = _CACHE["nc"]
    in_maps = host_prep(inputs)
    res = run_bass_kernel_spmd(nc, in_maps, list(range(8)))
    out = np.stack([r["out"] for r in res.results], axis=0)
    return out.astype(np.float32)
